# revision 1
# baseline (speedup 1.0000x reference)
"""Trainium2 Bass kernel for nn_MultiHeadMALAAttention.

Sharding: 8 cores; core c handles batch b = c//2, token half h = c%2
(tokens [h*4096, (h+1)*4096) of N=8192).  Stats (kmean, vmean, kv_state)
need full-N reductions -> pairwise AllReduce between the two cores of a
batch, replica groups [[0,1],[2,3],[4,5],[6,7]].

On-device layout: channel-major ("CT", [chan partitions, token free]) for
q/k/v/o/res; token-major transient tiles (via PE transpose) only for the
kv_state contraction over tokens.  All matmuls in bf16 (fp32 PSUM accum).

Host pre-work (part of sharding): transpose+cast x to bf16 channel-major
with a 1-token halo on each side (for the depthwise conv), replicate
sin/cos across the 4 heads of a 128-channel tile, pre-transpose/cast all
weights into lhsT layout, build the rotate-every-two block matrix, head
mask and identity constants.
"""

import os
import sys

sys.path.insert(0, "/opt/trn_rl_repo")

import numpy as np
import ml_dtypes

B, N, DIM, H, HD = 4, 8192, 256, 8, 32
INTERNAL = H * HD  # 256
SCALE = HD ** -0.5
NCORES = 8
T = N // 2          # tokens per core
TH = T + 2          # with 1-token halo each side
CH = 512            # chunk tokens
NCH = T // CH       # chunks per core
KSC = SCALE / N     # kv_state scale (s^2)

BF16 = ml_dtypes.bfloat16


# ---------------------------------------------------------------- host prep

def _host_prep(x, sin, cos, W_qkvo, b_qkvo, W_lepe, b_lepe, W_proj, b_proj):
    """Build per-core input dicts (all device tensors)."""
    WT = W_qkvo.T.astype(np.float32)          # [DIM, 1024] = lhsT layout
    wq = WT[:, 0:256].astype(BF16)
    wkv = WT[:, 256:768].astype(BF16)          # k cols 0:256, v cols 256:512
    wo = WT[:, 768:1024].astype(BF16)
    wp = W_proj.T.astype(np.float32).astype(BF16)   # [DIM, 256] rhs layout
    wl = W_lepe[:, 0, :].astype(np.float32)    # [256, 3]

    # diag conv weights: block (tap j, tile m) = diag(wl[128m:128(m+1), j])
    dcw = np.zeros((128, 6, 128), np.float32)
    for j in range(3):
        for m in range(2):
            np.fill_diagonal(dcw[:, j * 2 + m, :], wl[128 * m:128 * (m + 1), j])
    dcw = dcw.reshape(128, 768).astype(BF16)

    # rotate-every-two matrix as lhsT: rot = R.T @ x ; R[k, m] = coeff of
    # chan k in rot-chan m:  rot[2i] = -x[2i+1], rot[2i+1] = x[2i]
    R = np.zeros((128, 128), np.float32)
    for i in range(64):
        R[2 * i + 1, 2 * i] = -1.0
        R[2 * i, 2 * i + 1] = 1.0
    R = R.astype(BF16)

    hmask = np.zeros((128, 128), np.float32)
    for hh in range(4):
        hmask[32 * hh:32 * (hh + 1), 32 * hh:32 * (hh + 1)] = 1.0
    hmask = hmask.astype(BF16)

    ident16 = np.eye(128, dtype=np.float32).astype(BF16)
    ident32 = np.eye(128, dtype=np.float32)

    use_bias = bool(np.any(b_qkvo) or np.any(b_lepe) or np.any(b_proj))
    bqkvo = np.asarray(b_qkvo, np.float32).reshape(1, 1024).astype(BF16)
    blep = np.asarray(b_lepe, np.float32).reshape(1, 256).astype(BF16)
    bprj = np.asarray(b_proj, np.float32).reshape(1, 256).astype(BF16)

    xf = np.asarray(x, np.float32)
    sinf = np.asarray(sin, np.float32)
    cosf = np.asarray(cos, np.float32)

    per_core = []
    for c in range(NCORES):
        b = c // 2
        t0 = (c % 2) * T
        # x channel-major with halo [256, TH]
        xpad = np.zeros((TH, DIM), np.float32)
        lo, hi = t0 - 1, t0 + T + 1
        slo, shi = max(lo, 0), min(hi, N)
        xpad[slo - lo: slo - lo + (shi - slo)] = xf[b, slo:shi]
        xct = np.ascontiguousarray(xpad.T).astype(BF16)          # [256, TH]

        srep = np.tile(sinf[t0:t0 + T].T, (4, 1)).astype(BF16)   # [128, T]
        crep = np.tile(cosf[t0:t0 + T].T, (4, 1)).astype(BF16)   # [128, T]
        # paired layout: col = c*1024 + j*512 + t, same data for j=0,1
        srp = np.ascontiguousarray(np.broadcast_to(
            srep.reshape(128, NCH, 1, CH), (128, NCH, 2, CH)).reshape(128, 2 * T))
        crp = np.ascontiguousarray(np.broadcast_to(
            crep.reshape(128, NCH, 1, CH), (128, NCH, 2, CH)).reshape(128, 2 * T))

        per_core.append({
            "xct": xct, "srep": srep, "crep": crep, "srp": srp, "crp": crp,
            "wq": np.ascontiguousarray(wq), "wkv": np.ascontiguousarray(wkv),
            "wo": np.ascontiguousarray(wo), "wp": np.ascontiguousarray(wp),
            "dcw": dcw, "rblk": R, "hmask": hmask,
            "ident16": ident16, "ident32": ident32,
            "bqkvo": bqkvo, "blep": blep, "bprj": bprj,
        })
    return per_core, use_bias


# ------------------------------------------------------------ device kernel

def _build_nc(use_bias: bool, lvl: int = 3):
    from concourse import bacc
    import concourse.mybir as mybir
    import concourse.tile as tile

    dt = mybir.dt
    AF = mybir.ActivationFunctionType
    OP = mybir.AluOpType

    nc = bacc.Bacc(None, target_bir_lowering=False)

    # ---- I/O
    xct_d = nc.dram_tensor("xct", [256, TH], dt.bfloat16, kind="ExternalInput")
    # paired sin/cos, col = c*1024 + j*512 + t (chunk-interleaved for both tiles)
    srp_d = nc.dram_tensor("srp", [128, 2 * T], dt.bfloat16, kind="ExternalInput")
    crp_d = nc.dram_tensor("crp", [128, 2 * T], dt.bfloat16, kind="ExternalInput")
    wq_d = nc.dram_tensor("wq", [256, 256], dt.bfloat16, kind="ExternalInput")
    wkv_d = nc.dram_tensor("wkv", [256, 512], dt.bfloat16, kind="ExternalInput")
    wo_d = nc.dram_tensor("wo", [256, 256], dt.bfloat16, kind="ExternalInput")
    wp_d = nc.dram_tensor("wp", [256, 256], dt.bfloat16, kind="ExternalInput")
    dcw_d = nc.dram_tensor("dcw", [128, 768], dt.bfloat16, kind="ExternalInput")
    rblk_d = nc.dram_tensor("rblk", [128, 128], dt.bfloat16, kind="ExternalInput")
    hmask_d = nc.dram_tensor("hmask", [128, 128], dt.bfloat16, kind="ExternalInput")
    id16_d = nc.dram_tensor("ident16", [128, 128], dt.bfloat16, kind="ExternalInput")
    id32_d = nc.dram_tensor("ident32", [128, 128], dt.float32, kind="ExternalInput")
    bqkvo_d = nc.dram_tensor("bqkvo", [1, 1024], dt.bfloat16, kind="ExternalInput")
    blep_d = nc.dram_tensor("blep", [1, 256], dt.bfloat16, kind="ExternalInput")
    bprj_d = nc.dram_tensor("bprj", [1, 256], dt.bfloat16, kind="ExternalInput")
    out_d = nc.dram_tensor("out", [T, 256], dt.float32, kind="ExternalOutput")

    RG = [[0, 1], [2, 3], [4, 5], [6, 7]]
    P2 = 2 * CH  # paired free size 1024

    with tile.TileContext(nc) as tc:
        with (
            tc.tile_pool(name="const", bufs=1) as const,
            tc.tile_pool(name="work", bufs=2) as work,
            tc.tile_pool(name="psum", bufs=2, space="PSUM") as ppool,
            tc.tile_pool(name="pacc", bufs=1, space="PSUM") as pacc,
            tc.tile_pool(name="dram", bufs=1, space="DRAM") as dpool,
        ):
            def load(tname, dten, shape, dtype=dt.bfloat16):
                t_ = const.tile(shape, dtype, tag=tname, name=tname)
                nc.sync.dma_start(out=t_, in_=dten[:, :])
                return t_

            xct = [const.tile([128, TH], dt.bfloat16, tag=f"xct{k}", name=f"xct{k}")
                   for k in range(2)]
            for k in range(2):
                nc.sync.dma_start(out=xct[k], in_=xct_d[128 * k:128 * (k + 1), :])
            srp = load("srp", srp_d, [128, 2 * T])
            crp = load("crp", crp_d, [128, 2 * T])
            wq = [const.tile([128, 256], dt.bfloat16, tag=f"wq{k}", name=f"wq{k}")
                  for k in range(2)]
            wkv = [const.tile([128, 512], dt.bfloat16, tag=f"wkv{k}", name=f"wkv{k}")
                   for k in range(2)]
            wo = [const.tile([128, 256], dt.bfloat16, tag=f"wo{k}", name=f"wo{k}")
                  for k in range(2)]
            wp = [const.tile([128, 256], dt.bfloat16, tag=f"wp{k}", name=f"wp{k}")
                  for k in range(2)]
            for k in range(2):
                sl = slice(128 * k, 128 * (k + 1))
                nc.sync.dma_start(out=wq[k], in_=wq_d[sl, :])
                nc.sync.dma_start(out=wkv[k], in_=wkv_d[sl, :])
                nc.sync.dma_start(out=wo[k], in_=wo_d[sl, :])
                nc.sync.dma_start(out=wp[k], in_=wp_d[sl, :])
            dcw = load("dcw", dcw_d, [128, 768])
            rblk = load("rblk", rblk_d, [128, 128])
            hmask = load("hmask", hmask_d, [128, 128])
            id16 = load("id16", id16_d, [128, 128])
            id32 = load("id32", id32_d, [128, 128], dt.float32)
            ones = None
            if use_bias:
                bqkvo = load("bqkvo", bqkvo_d, [1, 1024])
                blep = load("blep", blep_d, [1, 256])
                bprj = load("bprj", bprj_d, [1, 256])
                ones = const.tile([1, CH], dt.bfloat16, tag="ones", name="ones")
                nc.vector.memset(ones, 1.0)

            # persistent activations (paired layout: col c*1024 + j*512 + t)
            q1p = const.tile([128, 2 * T], dt.bfloat16, tag="q1p", name="q1p")
            o1p = const.tile([128, 2 * T], dt.bfloat16, tag="o1p", name="o1p")
            vT = [const.tile([128, TH], dt.bfloat16, tag=f"vT{j}", name=f"vT{j}")
                  for j in range(2)]
            kpart = const.tile([128, 32], dt.float32, tag="kpart", name="kpart")
            vpart = const.tile([128, 16], dt.float32, tag="vpart", name="vpart")
            stats = const.tile([128, 260], dt.float32, tag="stats", name="stats")
            stats2 = const.tile([128, 260], dt.float32, tag="stats2", name="stats2")

            gram = pacc.tile([128, 256], dt.float32, tag="gram", name="gram")

            # =========================== phase 1 ===========================
            for c in range(NCH):
                xsl = [x[:, 1 + c * CH: 1 + (c + 1) * CH] for x in xct]
                psl = slice(c * P2, (c + 1) * P2)       # paired persists slice
                ssl = srp[:, psl]
                csl = crp[:, psl]

                # ---- q / k / v / o projections (paired psum, per-bank groups)
                qps = ppool.tile([128, P2], dt.float32, tag="w", name="qps")
                kps = ppool.tile([128, P2], dt.float32, tag="w", name="kps")
                vps = ppool.tile([128, P2], dt.float32, tag="w", name="vps")
                ops_ = ppool.tile([128, P2], dt.float32, tag="w", name="ops_")
                for j in range(2):
                    cols = slice(j * CH, (j + 1) * CH)
                    msl = slice(128 * j, 128 * (j + 1))
                    nc.tensor.matmul(qps[:, cols], wq[0][:, msl], xsl[0],
                                     start=True, stop=False)
                    nc.tensor.matmul(qps[:, cols], wq[1][:, msl], xsl[1],
                                     start=False, stop=not use_bias)
                    if use_bias:
                        nc.tensor.matmul(qps[:, cols], bqkvo[:, msl], ones,
                                         start=False, stop=True)
                    nc.tensor.matmul(kps[:, cols], wkv[0][:, msl], xsl[0],
                                     start=True, stop=False)
                    nc.tensor.matmul(kps[:, cols], wkv[1][:, msl], xsl[1],
                                     start=False, stop=not use_bias)
                    if use_bias:
                        nc.tensor.matmul(kps[:, cols],
                                         bqkvo[:, 256 + 128 * j:256 + 128 * (j + 1)],
                                         ones, start=False, stop=True)
                    vsl = slice(256 + 128 * j, 256 + 128 * (j + 1))
                    nc.tensor.matmul(vps[:, cols], wkv[0][:, vsl], xsl[0],
                                     start=True, stop=False)
                    nc.tensor.matmul(vps[:, cols], wkv[1][:, vsl], xsl[1],
                                     start=False, stop=not use_bias)
                    if use_bias:
                        nc.tensor.matmul(vps[:, cols],
                                         bqkvo[:, 512 + 128 * j:512 + 128 * (j + 1)],
                                         ones, start=False, stop=True)
                    nc.tensor.matmul(ops_[:, cols], wo[0][:, msl], xsl[0],
                                     start=True, stop=False)
                    nc.tensor.matmul(ops_[:, cols], wo[1][:, msl], xsl[1],
                                     start=False, stop=not use_bias)
                    if use_bias:
                        nc.tensor.matmul(ops_[:, cols],
                                         bqkvo[:, 768 + 128 * j:768 + 128 * (j + 1)],
                                         ones, start=False, stop=True)

                # ---- q elu+1 -> q1p (paired ops)
                rq = work.tile([128, P2], dt.bfloat16, tag="rq", name="rq")
                nc.scalar.activation(rq, qps, AF.Relu)
                mn = work.tile([128, P2], dt.bfloat16, tag="mn", name="mn")
                nc.vector.tensor_scalar_min(mn, qps, 0.0)
                eq = work.tile([128, P2], dt.bfloat16, tag="eq", name="eq")
                nc.scalar.activation(eq, mn, AF.Exp)
                nc.gpsimd.tensor_add(q1p[:, psl], eq, rq)

                # ---- o evac -> o1p
                nc.scalar.activation(o1p[:, psl], ops_, AF.Copy)

                # ---- k elu+1 (ksum rides ACT accums)
                rk = work.tile([128, P2], dt.bfloat16, tag="rk", name="rk")
                nc.scalar.activation(rk[:, 0:CH], kps[:, 0:CH], AF.Relu,
                                     accum_out=kpart[:, c:c + 1])
                nc.scalar.activation(rk[:, CH:P2], kps[:, CH:P2], AF.Relu,
                                     accum_out=kpart[:, 8 + c:9 + c])
                mnk = work.tile([128, P2], dt.bfloat16, tag="mnk", name="mnk")
                nc.vector.tensor_scalar_min(mnk, kps, 0.0)
                ek = work.tile([128, P2], dt.bfloat16, tag="ek", name="ek")
                nc.scalar.activation(ek[:, 0:CH], mnk[:, 0:CH], AF.Exp,
                                     accum_out=kpart[:, 16 + c:17 + c])
                nc.scalar.activation(ek[:, CH:P2], mnk[:, CH:P2], AF.Exp,
                                     accum_out=kpart[:, 24 + c:25 + c])
                k1t = work.tile([128, P2], dt.bfloat16, tag="k1t", name="k1t")
                nc.vector.tensor_add(k1t, ek, rk)

                # ---- v evac -> vT (vsum rides ACT accums)
                for j in range(2):
                    nc.scalar.activation(
                        vT[j][:, 1 + c * CH: 1 + (c + 1) * CH],
                        vps[:, j * CH:(j + 1) * CH], AF.Copy,
                        accum_out=vpart[:, 8 * j + c: 8 * j + c + 1])

                # ---- K rope (one paired rot MM)
                rkp = ppool.tile([128, P2], dt.float32, tag="w", name="rkp")
                nc.tensor.matmul(rkp[:, 0:CH], rblk, k1t[:, 0:CH],
                                 start=True, stop=True)
                nc.tensor.matmul(rkp[:, CH:P2], rblk, k1t[:, CH:P2],
                                 start=True, stop=True)
                m1 = work.tile([128, P2], dt.bfloat16, tag="m1", name="m1")
                nc.gpsimd.tensor_mul(m1, k1t, csl)
                m2 = work.tile([128, P2], dt.bfloat16, tag="m2", name="m2")
                nc.vector.tensor_mul(m2, rkp, ssl)
                ks = work.tile([128, P2], dt.bfloat16, tag="ks", name="ks")
                nc.vector.tensor_add(ks, m1, m2)

                # ---- transpose ks, v to token-major; kv gram accumulation
                for s in range(4):
                    ktp = ppool.tile([128, CH], dt.bfloat16, tag="tp", bufs=2,
                                     name="ktp")
                    nc.tensor.transpose(ktp[:, 0:128],
                                        ks[:, s * 128:(s + 1) * 128], id16)
                    nc.tensor.transpose(ktp[:, 128:256],
                                        ks[:, CH + s * 128:CH + (s + 1) * 128], id16)
                    vcol = 1 + c * CH + s * 128
                    nc.tensor.transpose(ktp[:, 256:384],
                                        vT[0][:, vcol:vcol + 128], id16)
                    nc.tensor.transpose(ktp[:, 384:512],
                                        vT[1][:, vcol:vcol + 128], id16)
                    kvtok = work.tile([128, CH], dt.bfloat16, tag="kvtok",
                                      name="kvtok")
                    if s % 2 == 0:
                        nc.scalar.activation(kvtok, ktp, AF.Copy)
                    else:
                        nc.vector.tensor_copy(kvtok, ktp)
                    first = (c == 0 and s == 0)
                    last = (c == NCH - 1 and s == 3)
                    nc.tensor.matmul(gram[:, 0:128], kvtok[:, 0:128],
                                     kvtok[:, 256:384], start=first, stop=False)
                    nc.tensor.matmul(gram[:, 128:256], kvtok[:, 128:256],
                                     kvtok[:, 384:512], start=False, stop=last)

            # ---- halo v columns (tokens t0-1 and t0+T) for the conv
            vhp = ppool.tile([128, CH], dt.float32, tag="tp", bufs=2, name="vhp")
            for j in range(2):
                vsl = slice(256 + 128 * j, 256 + 128 * (j + 1))
                cl = slice(j * 4, j * 4 + 1)
                cr = slice(j * 4 + 2, j * 4 + 3)
                nc.tensor.matmul(vhp[:, cl], wkv[0][:, vsl], xct[0][:, 0:1],
                                 start=(j == 0), stop=False)
                nc.tensor.matmul(vhp[:, cl], wkv[1][:, vsl], xct[1][:, 0:1],
                                 start=False, stop=False)
                nc.tensor.matmul(vhp[:, cr], wkv[0][:, vsl], xct[0][:, TH - 1:TH],
                                 start=False, stop=False)
                nc.tensor.matmul(vhp[:, cr], wkv[1][:, vsl], xct[1][:, TH - 1:TH],
                                 start=False, stop=(j == 1))
            for j in range(2):
                nc.scalar.activation(vT[j][:, 0:1], vhp[:, j * 4:j * 4 + 1], AF.Copy)
                nc.scalar.activation(vT[j][:, TH - 1:TH],
                                     vhp[:, j * 4 + 2:j * 4 + 3], AF.Copy)

            # ======================= stats + allreduce =====================
            nc.vector.tensor_scalar_mul(stats[:, 0:256], gram, 1.0)
            nc.vector.tensor_reduce(stats[:, 256:257], kpart[:, 0:8],
                                    axis=mybir.AxisListType.X, op=OP.add)
            nc.vector.tensor_reduce(stats[:, 257:258], kpart[:, 8:16],
                                    axis=mybir.AxisListType.X, op=OP.add)
            kx = const.tile([128, 2], dt.float32, tag="kx", name="kx")
            nc.vector.tensor_reduce(kx[:, 0:1], kpart[:, 16:24],
                                    axis=mybir.AxisListType.X, op=OP.add)
            nc.vector.tensor_reduce(kx[:, 1:2], kpart[:, 24:32],
                                    axis=mybir.AxisListType.X, op=OP.add)
            nc.vector.tensor_add(stats[:, 256:258], stats[:, 256:258], kx)
            nc.vector.tensor_reduce(stats[:, 258:259], vpart[:, 0:8],
                                    axis=mybir.AxisListType.X, op=OP.add)
            nc.vector.tensor_reduce(stats[:, 259:260], vpart[:, 8:16],
                                    axis=mybir.AxisListType.X, op=OP.add)

            if lvl >= 2 and os.environ.get("KERNEL_NOCC"):
                nc.vector.tensor_scalar_mul(stats2, stats, 1.0)
            elif lvl >= 2:
                ccin = dpool.tile([128, 260], dt.float32, tag="ccin", name="ccin")
                ccout = dpool.tile([128, 260], dt.float32, tag="ccout", name="ccout")
                nc.gpsimd.dma_start(out=ccin[:, :], in_=stats)
                nc.gpsimd.collective_compute(
                    "AllReduce", OP.add, replica_groups=RG,
                    ins=[ccin[:, :]], outs=[ccout[:, :]])
                nc.gpsimd.dma_start(out=stats2, in_=ccout[:, :])
            else:
                nc.vector.tensor_scalar_mul(stats2, stats, 1.0)

            if lvl <= 2:
                for c in range(NCH):
                    dummy = work.tile([128, CH], dt.float32, tag="outsb",
                                      name="dummy")
                    nc.vector.tensor_scalar_mul(
                        dummy, stats2[:, 0:1].to_broadcast((128, CH)), 1.0)
                    for h in range(2):
                        dsl = out_d[c * CH + h * 256: c * CH + (h + 1) * 256, :]
                        nc.sync.dma_start(
                            out=dsl.rearrange("(s t) o -> t s o", s=2), in_=dummy)

            if lvl >= 3:
                _phase2(locals())

    nc.compile()
    return nc


def _phase2(env):
    nc = env["nc"]; dt = env["dt"]; AF = env["AF"]; OP = env["OP"]
    const = env["const"]; work = env["work"]; ppool = env["ppool"]
    use_bias = env["use_bias"]; stats2 = env["stats2"]; hmask = env["hmask"]
    id32 = env["id32"]; srp = env["srp"]; crp = env["crp"]
    wp = env["wp"]; dcw = env["dcw"]; rblk = env["rblk"]
    q1p = env["q1p"]; o1p = env["o1p"]; vT = env["vT"]; out_d = env["out_d"]
    ones = env["ones"]
    P2 = 2 * CH
    if use_bias:
        blep = env["blep"]; bprj = env["bprj"]
    import concourse.mybir as mybir

    # ---- assemble small matrices
    zsc = const.tile([128, 2], dt.float32, tag="zsc", name="zsc")
    nc.scalar.mul(zsc[:, 0:1], stats2[:, 256:257], SCALE / N)
    nc.scalar.mul(zsc[:, 1:2], stats2[:, 257:258], SCALE / N)

    zblk = []
    mcorr = []
    kvblk = []
    for j in range(2):
        zb = const.tile([128, 128], dt.bfloat16, tag=f"zblk{j}", name=f"zblk{j}")
        nc.vector.tensor_tensor(
            zb, zsc[:, j:j + 1].to_broadcast((128, 128)), hmask, OP.mult)
        zblk.append(zb)

        vrp = ppool.tile([128, CH], dt.float32, tag="tp", bufs=2, name="vrp")
        nc.tensor.transpose(vrp[0:1, 0:128], stats2[:, 258 + j:259 + j], id32)
        vrow = const.tile([1, 128], dt.float32, tag=f"vrow{j}", name=f"vrow{j}")
        nc.scalar.mul(vrow, vrp[0:1, 0:128], -1.0 / N)
        vrowb = const.tile([128, 128], dt.float32, tag=f"vrowb{j}", name=f"vrowb{j}")
        nc.gpsimd.partition_broadcast(vrowb, vrow)
        mc0 = const.tile([128, 128], dt.float32, tag=f"mc0{j}", name=f"mc0{j}")
        nc.vector.tensor_tensor(
            mc0, zsc[:, j:j + 1].to_broadcast((128, 128)), vrowb, OP.mult)
        mc = const.tile([128, 128], dt.bfloat16, tag=f"mc{j}", name=f"mc{j}")
        nc.vector.tensor_tensor(mc, mc0, hmask, OP.mult)
        mcorr.append(mc)

        kvb = const.tile([128, 128], dt.bfloat16, tag=f"kvb{j}", name=f"kvb{j}")
        nc.vector.memset(kvb, 0.0)
        for a in range(4):
            psl = slice(32 * a, 32 * (a + 1))
            nc.scalar.mul(kvb[psl, psl],
                          stats2[psl, 128 * j + 32 * a: 128 * j + 32 * (a + 1)],
                          KSC)
        kvblk.append(kvb)

    # =========================== phase 2 ===========================
    for c in range(NCH):
        psl = slice(c * P2, (c + 1) * P2)
        ssl = srp[:, psl]
        csl = crp[:, psl]
        q1sl = q1p[:, psl]

        zps = ppool.tile([128, P2], dt.float32, tag="w", name="zps")
        nc.tensor.matmul(zps[:, 0:CH], zblk[0], q1p[:, c * P2:c * P2 + CH],
                         start=True, stop=True)
        nc.tensor.matmul(zps[:, CH:P2], zblk[1], q1p[:, c * P2 + CH:(c + 1) * P2],
                         start=True, stop=True)
        rz = work.tile([128, P2], dt.float32, tag="rz", name="rz")
        nc.vector.reciprocal_approx_fast(out=rz, in_=zps)
        qa = work.tile([128, P2], dt.bfloat16, tag="qa", name="qa")
        nc.vector.scalar_tensor_tensor(out=qa, in0=rz, scalar=1.0, in1=q1sl,
                                       op0=OP.add, op1=OP.mult)
        rqp = ppool.tile([128, P2], dt.float32, tag="w", name="rqp")
        nc.tensor.matmul(rqp[:, 0:CH], rblk, qa[:, 0:CH], start=True, stop=True)
        nc.tensor.matmul(rqp[:, CH:P2], rblk, qa[:, CH:P2], start=True, stop=True)
        t1 = work.tile([128, P2], dt.bfloat16, tag="t1", name="t1")
        nc.gpsimd.tensor_mul(t1, qa, csl)
        t2 = work.tile([128, P2], dt.bfloat16, tag="t2", name="t2")
        nc.vector.tensor_mul(t2, rqp, ssl)

        rps = ppool.tile([128, P2], dt.float32, tag="w", name="rps")
        for j in range(2):
            cols = slice(j * CH, (j + 1) * CH)
            nc.tensor.matmul(rps[:, cols], kvblk[j], t1[:, cols],
                             start=True, stop=False)
            nc.tensor.matmul(rps[:, cols], kvblk[j], t2[:, cols],
                             start=False, stop=False)
            nc.tensor.matmul(rps[:, cols], mcorr[j],
                             q1p[:, c * P2 + j * CH: c * P2 + (j + 1) * CH],
                             start=False, stop=False)
            for tap in range(3):
                lastmm = (tap == 2 and not use_bias)
                nc.tensor.matmul(
                    rps[:, cols],
                    dcw[:, (tap * 2 + j) * 128:(tap * 2 + j + 1) * 128],
                    vT[j][:, c * CH + tap: c * CH + tap + CH],
                    start=False, stop=lastmm)
            if use_bias:
                nc.tensor.matmul(rps[:, cols], blep[:, 128 * j:128 * (j + 1)],
                                 ones, start=False, stop=True)

        y = work.tile([128, P2], dt.bfloat16, tag="y", name="y")
        nc.vector.tensor_mul(y, rps, o1p[:, psl])

        outp = ppool.tile([128, P2], dt.float32, tag="w", name="outp")
        for h in range(2):
            for si in range(2):
                s = h * 2 + si
                osl = slice(s * 256, (s + 1) * 256)
                first = (si == 0)
                nc.tensor.matmul(outp[:, osl], y[:, s * 128:(s + 1) * 128],
                                 wp[0], start=first, stop=False)
                last = (si == 1 and not use_bias)
                nc.tensor.matmul(outp[:, osl],
                                 y[:, CH + s * 128:CH + (s + 1) * 128],
                                 wp[1], start=False, stop=last)
                if use_bias:
                    nc.tensor.matmul(outp[:, osl], ones[:, 0:128], bprj,
                                     start=False, stop=(si == 1))
        outsb = work.tile([128, P2], dt.float32, tag="outsb", name="outsb")
        nc.scalar.activation(outsb, outp, AF.Copy)
        dsl = out_d[c * CH: (c + 1) * CH, :]
        nc.sync.dma_start(out=dsl.rearrange("(s t) o -> t s o", s=4), in_=outsb)



_NC_CACHE = {}


def _get_nc(use_bias: bool):
    lvl = int(os.environ.get("KERNEL_LVL", "3"))
    key = (use_bias, lvl)
    if key not in _NC_CACHE:
        _NC_CACHE[key] = _build_nc(use_bias, lvl)
    return _NC_CACHE[key]


def kernel(x, sin, cos, W_qkvo, b_qkvo, W_lepe, b_lepe, W_proj, b_proj):
    from concourse.bass_utils import run_bass_kernel_spmd

    per_core, use_bias = _host_prep(x, sin, cos, W_qkvo, b_qkvo, W_lepe,
                                    b_lepe, W_proj, b_proj)
    nc = _get_nc(use_bias)
    # keep only the inputs that survived DCE in the compiled program
    import concourse.mybir as mybir
    expected = set()
    for alloc in nc.m.functions[0].allocations:
        if isinstance(alloc, mybir.MemoryLocationSet) and alloc.kind == "ExternalInput":
            expected.add(alloc.memorylocations[0].name)
    per_core = [{k: v for k, v in m.items() if k in expected} for m in per_core]
    res = run_bass_kernel_spmd(nc, per_core, core_ids=list(range(NCORES)),
                               trace=bool(os.environ.get("KERNEL_TRACE")))
    if os.environ.get("KERNEL_TRACE"):
        kernel.last_exec_time_ns = res.exec_time_ns
        kernel.last_results = res
    full = np.zeros((B, N, INTERNAL), np.float32)
    for c in range(NCORES):
        b = c // 2
        t0 = (c % 2) * T
        full[b, t0:t0 + T] = res.results[c]["out"]
    return full


# ---------------------------------------------------------- numpy reference
# A numpy emulation of the exact device pipeline (fp32), used to validate
# the decomposition (run with KERNEL_SELFTEST=1).

def _numpy_pipeline(per_core_inputs, skip_pair=False):
    outs = []
    cores = []
    for c in range(NCORES):
        d = per_core_inputs[c]
        xct = d["xct"].astype(np.float32)          # [256, TH]
        srep = d["srep"].astype(np.float32)
        crep = d["crep"].astype(np.float32)
        wq = d["wq"].astype(np.float32)
        wkv = d["wkv"].astype(np.float32)
        wo = d["wo"].astype(np.float32)
        wp = d["wp"].astype(np.float32)
        dcw = d["dcw"].astype(np.float32).reshape(128, 6, 128)
        R = d["rblk"].astype(np.float32)
        hmask = d["hmask"].astype(np.float32)

        x_in = xct[:, 1:T + 1]                     # [256, T]
        qT = wq.T @ x_in                           # [256, T]
        kT = wkv[:, 0:256].T @ x_in
        vT_m = wkv[:, 256:512].T @ x_in
        oT = wo.T @ x_in
        # halo v cols
        vhl = wkv[:, 256:512].T @ xct[:, 0:1]
        vhr = wkv[:, 256:512].T @ xct[:, TH - 1:TH]
        vT = np.concatenate([vhl, vT_m, vhr], axis=1)      # [256, TH]

        def elu1(t):
            return np.exp(np.minimum(t, 0.0)) + np.maximum(t, 0.0)

        q1 = elu1(qT)
        k1 = elu1(kT)

        # K rope (per chan-tile with R)
        ks = np.zeros_like(k1)
        for j in range(2):
            blk = k1[128 * j:128 * (j + 1)]
            ks[128 * j:128 * (j + 1)] = blk * crep + (R.T @ blk) * srep

        # kv gram per tile: ks_j^T tokens x v_j
        gram = np.zeros((128, 256), np.float32)
        for j in range(2):
            gram[:, 128 * j:128 * (j + 1)] = (
                ks[128 * j:128 * (j + 1)] @ vT[128 * j:128 * (j + 1), 1:T + 1].T)
        ksum = k1.sum(axis=1)                      # [256]
        vsum = vT[:, 1:T + 1].sum(axis=1)
        cores.append(dict(d=d, q1=q1, oT=oT, vT=vT, gram=gram, ksum=ksum,
                          vsum=vsum, R=R, hmask=hmask, dcw=dcw, wp=wp,
                          srep=srep, crep=crep))

    for pair in range(4):
        a, b2 = cores[2 * pair], cores[2 * pair + 1]
        if skip_pair:
            for cc in (a, b2):
                cc["gram_r"], cc["ksum_r"], cc["vsum_r"] = (
                    cc["gram"], cc["ksum"], cc["vsum"])
            continue
        gram = a["gram"] + b2["gram"]
        ksum = a["ksum"] + b2["ksum"]
        vsum = a["vsum"] + b2["vsum"]
        for cc in (a, b2):
            cc["gram_r"], cc["ksum_r"], cc["vsum_r"] = gram, ksum, vsum

    for c in range(NCORES):
        st = cores[c]
        q1, oT, vT = st["q1"], st["oT"], st["vT"]
        R, hmask, dcw, wp = st["R"], st["hmask"], st["dcw"], st["wp"]
        srep, crep = st["srep"], st["crep"]
        gram, ksum, vsum = st["gram_r"], st["ksum_r"], st["vsum_r"]

        kmean = ksum / N
        vmean = vsum / N
        out = np.zeros((T, 256), np.float32)
        res = np.zeros((256, T), np.float32)
        for j in range(2):
            sl = slice(128 * j, 128 * (j + 1))
            zsc = SCALE * kmean[sl]                          # [128]
            zblk = (zsc[:, None] * hmask)                    # [128,128]
            zrep = zblk.T @ q1[sl]                           # [128, T]
            r = 1.0 / zrep
            qa = q1[sl] * (1.0 + r)
            t1 = qa * crep
            t2 = (R.T @ qa) * srep
            kvblk = np.zeros((128, 128), np.float32)
            for aa in range(4):
                s2 = slice(32 * aa, 32 * (aa + 1))
                kvblk[s2, s2] = KSC * gram[s2, 128 * j + 32 * aa:128 * j + 32 * (aa + 1)]
            mcorr = -(zsc[:, None]) * (vmean[sl][None, :] / 1.0) * hmask / 1.0
            mcorr = mcorr * 1.0
            # note: corr = z (x) vmean -> M[k, c] = SCALE*kmean[k]*vmean[c]*mask
            lepe = np.zeros((128, T), np.float32)
            for tap in range(3):
                dw = dcw[:, tap * 2 + j, :]
                lepe += dw.T @ vT[sl, tap:tap + T]
            res[sl] = (kvblk.T @ t1 + kvblk.T @ t2 + mcorr.T @ q1[sl] + lepe)
        y = res * oT
        out = y.T @ wp            # wait: out[t, oc] = sum_c y[c,t] wp[c,oc]
        outs.append(out.astype(np.float32))

    # unshard
    full = np.zeros((B, N, 256), np.float32)
    for c in range(NCORES):
        b = c // 2
        t0 = (c % 2) * T
        full[b, t0:t0 + T] = outs[c]
    return full


if __name__ == "__main__" and os.environ.get("KERNEL_BUILD"):
    nc = _build_nc(False)
    import tempfile
    from concourse.bass_utils import compile_bass_kernel
    print("NEFF:", compile_bass_kernel(nc, tempfile.mkdtemp()))

if __name__ == "__main__" and os.environ.get("KERNEL_SELFTEST"):
    sys.path.insert(0, os.path.dirname(os.path.abspath(__file__)))
    import reference
    inputs = reference.setup_inputs()
    inputs = {k: np.asarray(v) for k, v in inputs.items()}
    expected = np.asarray(reference.reference(**inputs))
    per_core, use_bias = _host_prep(**inputs)
    got = _numpy_pipeline(per_core)
    err = np.abs(got - expected)
    rel = np.linalg.norm(got - expected) / np.linalg.norm(expected)
    print("selftest rel err:", rel, "max abs:", err.max())



# revision 7
# speedup vs baseline: 1.4490x; 1.4490x over previous
"""Trainium2 Bass kernel for nn_MultiHeadMALAAttention.

Sharding: 8 cores; core c handles batch b = c//2, token half h = c%2
(tokens [h*4096, (h+1)*4096) of N=8192).  Stats (kmean, vmean, kv_state)
need full-N reductions -> pairwise AllReduce between the two cores of a
batch, replica groups [[0,1],[2,3],[4,5],[6,7]].

Pipeline structure (v2 — restructured for overlap):
  phase K : k/v projections, elu(k)+1, rope(k), transposes + kv-gram,
            ksum/vsum accumulation.          (before the collective)
  AllReduce of [gram | ksum | vsum]  (133 KB, latency-bound ~25us)
  phase Q : q/o projections, elu(q)+1, rope(q) -> qs0.  Emitted after
            the collective start; no gpsimd use, so it runs *during*
            the collective.
  phase Z : small stats post-processing (zsc/zblk/kvblk/mcorr).
  phase C : z matmul, qa=(1+1/z)*qs0, attn+lepe+corr fused in PSUM,
            y=res*o, output projection, bf16 store.

On-device layout: channel-major ("CT", [chan partitions, token free])
throughout; token-major transient tiles (PE transpose) only for the
kv-gram contraction.  All matmuls bf16 (fp32 PSUM accum).
"""

import os
import sys

sys.path.insert(0, "/opt/trn_rl_repo")

import numpy as np
import ml_dtypes

B, N, DIM, H, HD = 4, 8192, 256, 8, 32
INTERNAL = H * HD  # 256
SCALE = HD ** -0.5
NCORES = 8
T = N // 2          # tokens per core
TH = T + 2          # with 1-token halo each side
CH = 512            # chunk tokens
NCH = T // CH       # chunks per core
KSC = SCALE / N     # kv_state scale (s^2)
P2 = 2 * CH         # paired free size 1024

BF16 = ml_dtypes.bfloat16


# ---------------------------------------------------------------- host prep

def _host_prep(x, sin, cos, W_qkvo, b_qkvo, W_lepe, b_lepe, W_proj, b_proj):
    """Build per-core input dicts (all device tensors)."""
    WT = W_qkvo.T.astype(np.float32)          # [DIM, 1024] = lhsT layout
    wq = WT[:, 0:256].astype(BF16)
    wkv = WT[:, 256:768].astype(BF16)          # k cols 0:256, v cols 256:512
    wo = WT[:, 768:1024].astype(BF16)
    wp = W_proj.T.astype(np.float32).astype(BF16)   # [DIM, 256] rhs layout
    wl = W_lepe[:, 0, :].astype(np.float32)    # [256, 3]

    # diag conv weights: block (tap j, tile m) = diag(wl[128m:128(m+1), j])
    dcw = np.zeros((128, 6, 128), np.float32)
    for j in range(3):
        for m in range(2):
            np.fill_diagonal(dcw[:, j * 2 + m, :], wl[128 * m:128 * (m + 1), j])
    dcw = dcw.reshape(128, 768).astype(BF16)

    # rotate-every-two matrix as lhsT: rot = R.T @ x ; R[k, m] = coeff of
    # chan k in rot-chan m:  rot[2i] = -x[2i+1], rot[2i+1] = x[2i]
    R = np.zeros((128, 128), np.float32)
    for i in range(64):
        R[2 * i + 1, 2 * i] = -1.0
        R[2 * i, 2 * i + 1] = 1.0
    R = R.astype(BF16)

    hmask = np.zeros((128, 128), np.float32)
    for hh in range(4):
        hmask[32 * hh:32 * (hh + 1), 32 * hh:32 * (hh + 1)] = 1.0
    hmk = (hmask * KSC).astype(BF16)           # pre-scaled mask for kvblk
    hmask = hmask.astype(BF16)

    ident16 = np.eye(128, dtype=np.float32).astype(BF16)

    use_bias = bool(np.any(b_qkvo) or np.any(b_lepe) or np.any(b_proj))
    bqkvo = np.asarray(b_qkvo, np.float32).reshape(1, 1024).astype(BF16)
    blep = np.asarray(b_lepe, np.float32).reshape(1, 256).astype(BF16)
    bprj = np.asarray(b_proj, np.float32).reshape(1, 256).astype(BF16)

    xf = np.asarray(x, np.float32)
    sinf = np.asarray(sin, np.float32)
    cosf = np.asarray(cos, np.float32)

    per_core = []
    for c in range(NCORES):
        b = c // 2
        t0 = (c % 2) * T
        # x channel-major with halo [256, TH]
        xpad = np.zeros((TH, DIM), np.float32)
        lo, hi = t0 - 1, t0 + T + 1
        slo, shi = max(lo, 0), min(hi, N)
        xpad[slo - lo: slo - lo + (shi - slo)] = xf[b, slo:shi]
        xct = np.ascontiguousarray(xpad.T).astype(BF16)          # [256, TH]

        srep = np.tile(sinf[t0:t0 + T].T, (4, 1)).astype(BF16)   # [128, T]
        crep = np.tile(cosf[t0:t0 + T].T, (4, 1)).astype(BF16)   # [128, T]

        per_core.append({
            "xct": xct, "srep": np.ascontiguousarray(srep),
            "crep": np.ascontiguousarray(crep),
            "wq": np.ascontiguousarray(wq), "wkv": np.ascontiguousarray(wkv),
            "wo": np.ascontiguousarray(wo), "wp": np.ascontiguousarray(wp),
            "dcw": dcw, "rblk": R, "hmask": hmask, "hmk": hmk,
            "ident16": ident16,
            "bqkvo": bqkvo, "blep": blep, "bprj": bprj,
        })
    return per_core, use_bias


# ------------------------------------------------------------ device kernel

def _build_nc(use_bias: bool, nocc: bool = False):
    from concourse import bacc
    import concourse.mybir as mybir
    import concourse.tile as tile

    dt = mybir.dt
    AF = mybir.ActivationFunctionType
    OP = mybir.AluOpType

    nc = bacc.Bacc(None, target_bir_lowering=False)

    # ---- I/O
    xct_d = nc.dram_tensor("xct", [256, TH], dt.bfloat16, kind="ExternalInput")
    srep_d = nc.dram_tensor("srep", [128, T], dt.bfloat16, kind="ExternalInput")
    crep_d = nc.dram_tensor("crep", [128, T], dt.bfloat16, kind="ExternalInput")
    wq_d = nc.dram_tensor("wq", [256, 256], dt.bfloat16, kind="ExternalInput")
    wkv_d = nc.dram_tensor("wkv", [256, 512], dt.bfloat16, kind="ExternalInput")
    wo_d = nc.dram_tensor("wo", [256, 256], dt.bfloat16, kind="ExternalInput")
    wp_d = nc.dram_tensor("wp", [256, 256], dt.bfloat16, kind="ExternalInput")
    dcw_d = nc.dram_tensor("dcw", [128, 768], dt.bfloat16, kind="ExternalInput")
    rblk_d = nc.dram_tensor("rblk", [128, 128], dt.bfloat16, kind="ExternalInput")
    hmask_d = nc.dram_tensor("hmask", [128, 128], dt.bfloat16, kind="ExternalInput")
    hmk_d = nc.dram_tensor("hmk", [128, 128], dt.bfloat16, kind="ExternalInput")
    id16_d = nc.dram_tensor("ident16", [128, 128], dt.bfloat16, kind="ExternalInput")
    bqkvo_d = nc.dram_tensor("bqkvo", [1, 1024], dt.bfloat16, kind="ExternalInput")
    blep_d = nc.dram_tensor("blep", [1, 256], dt.bfloat16, kind="ExternalInput")
    bprj_d = nc.dram_tensor("bprj", [1, 256], dt.bfloat16, kind="ExternalInput")
    out_d = nc.dram_tensor("out", [T, 256], dt.bfloat16, kind="ExternalOutput")

    RG = [[0, 1], [2, 3], [4, 5], [6, 7]]

    with tile.TileContext(nc) as tc:
        with (
            tc.tile_pool(name="const", bufs=1) as const,
            tc.tile_pool(name="work", bufs=2) as work,
            tc.tile_pool(name="psum", bufs=2, space="PSUM") as ppool,
            tc.tile_pool(name="pacc", bufs=1, space="PSUM") as pacc,
            tc.tile_pool(name="dram", bufs=1, space="DRAM") as dpool,
        ):
            def load(tname, dten, shape, dtype=dt.bfloat16):
                t_ = const.tile(shape, dtype, tag=tname, name=tname)
                nc.sync.dma_start(out=t_, in_=dten[:, :])
                return t_

            # weights/constants first (small, needed by chunk 0)
            wq = [const.tile([128, 256], dt.bfloat16, tag=f"wq{k}", name=f"wq{k}")
                  for k in range(2)]
            wkv = [const.tile([128, 512], dt.bfloat16, tag=f"wkv{k}", name=f"wkv{k}")
                   for k in range(2)]
            wo = [const.tile([128, 256], dt.bfloat16, tag=f"wo{k}", name=f"wo{k}")
                  for k in range(2)]
            wp = [const.tile([128, 256], dt.bfloat16, tag=f"wp{k}", name=f"wp{k}")
                  for k in range(2)]
            for k in range(2):
                sl = slice(128 * k, 128 * (k + 1))
                nc.sync.dma_start(out=wq[k], in_=wq_d[sl, :])
                nc.sync.dma_start(out=wkv[k], in_=wkv_d[sl, :])
                nc.sync.dma_start(out=wo[k], in_=wo_d[sl, :])
                nc.sync.dma_start(out=wp[k], in_=wp_d[sl, :])
            dcw = load("dcw", dcw_d, [128, 768])
            rblk = load("rblk", rblk_d, [128, 128])
            hmask = load("hmask", hmask_d, [128, 128])
            hmk = load("hmk", hmk_d, [128, 128])
            id16 = load("id16", id16_d, [128, 128])
            ones = None
            if use_bias:
                bqkvo = load("bqkvo", bqkvo_d, [1, 1024])
                blep = load("blep", blep_d, [1, 256])
                bprj = load("bprj", bprj_d, [1, 256])
                ones = const.tile([1, CH], dt.bfloat16, tag="ones", name="ones")
                nc.vector.memset(ones, 1.0)

            # chunked input loads (compute starts after the first pieces)
            xct = [const.tile([128, TH], dt.bfloat16, tag=f"xct{k}",
                              name=f"xct{k}") for k in range(2)]
            xcut = [0, 1025, 2049, 3073, TH]
            for p in range(4):
                for k in range(2):
                    nc.sync.dma_start(
                        out=xct[k][:, xcut[p]:xcut[p + 1]],
                        in_=xct_d[128 * k:128 * (k + 1), xcut[p]:xcut[p + 1]])
            srep = const.tile([128, T], dt.bfloat16, tag="srep", name="srep")
            crep = const.tile([128, T], dt.bfloat16, tag="crep", name="crep")
            for p in range(4):
                sl = slice(p * 1024, (p + 1) * 1024)
                nc.sync.dma_start(out=srep[:, sl], in_=srep_d[:, sl])
                nc.sync.dma_start(out=crep[:, sl], in_=crep_d[:, sl])

            # persistent activations (paired layout: col c*1024 + j*512 + t)
            q1p = const.tile([128, 2 * T], dt.bfloat16, tag="q1p", name="q1p")
            qs0 = const.tile([128, 2 * T], dt.bfloat16, tag="qs0", name="qs0")
            o1p = const.tile([128, 2 * T], dt.bfloat16, tag="o1p", name="o1p")
            vT = [const.tile([128, TH], dt.bfloat16, tag=f"vT{j}", name=f"vT{j}")
                  for j in range(2)]
            kpart = const.tile([128, 16], dt.float32, tag="kpart", name="kpart")
            vpart = const.tile([128, 16], dt.float32, tag="vpart", name="vpart")
            stats = const.tile([128, 260], dt.float32, tag="stats", name="stats")
            stats2 = const.tile([128, 260], dt.float32, tag="stats2",
                                name="stats2")

            gram = pacc.tile([128, 256], dt.float32, tag="gram", name="gram")

            # =========================== phase K ===========================
            # k/v projections, elu(k)+1, rope(k), transposes + gram, stats.
            for c in range(NCH):
                xsl = [x[:, 1 + c * CH: 1 + (c + 1) * CH] for x in xct]
                ssl = srep[:, c * CH:(c + 1) * CH]
                csl = crep[:, c * CH:(c + 1) * CH]

                kps = ppool.tile([128, P2], dt.float32, tag="big", name="kps")
                vps = ppool.tile([128, P2], dt.float32, tag="big", name="vps")
                for j in range(2):
                    cols = slice(j * CH, (j + 1) * CH)
                    ksl = slice(128 * j, 128 * (j + 1))
                    vsl = slice(256 + 128 * j, 256 + 128 * (j + 1))
                    nc.tensor.matmul(kps[:, cols], wkv[0][:, ksl], xsl[0],
                                     start=True, stop=False)
                    nc.tensor.matmul(kps[:, cols], wkv[1][:, ksl], xsl[1],
                                     start=False, stop=not use_bias)
                    if use_bias:
                        nc.tensor.matmul(kps[:, cols],
                                         bqkvo[:, 256 + 128 * j:256 + 128 * (j + 1)],
                                         ones, start=False, stop=True)
                    nc.tensor.matmul(vps[:, cols], wkv[0][:, vsl], xsl[0],
                                     start=True, stop=False)
                    nc.tensor.matmul(vps[:, cols], wkv[1][:, vsl], xsl[1],
                                     start=False, stop=not use_bias)
                    if use_bias:
                        nc.tensor.matmul(vps[:, cols],
                                         bqkvo[:, 512 + 128 * j:512 + 128 * (j + 1)],
                                         ones, start=False, stop=True)

                # elu(k)+1 : exp(min(k,0)) = min(exp(k),1);  +relu(k)
                ek = work.tile([128, P2], dt.bfloat16, tag="ek", name="ek")
                nc.scalar.activation(ek, kps, AF.Exp)
                mnk = work.tile([128, P2], dt.bfloat16, tag="mnk", name="mnk")
                nc.vector.tensor_scalar_min(mnk, ek, 1.0)
                k1t = work.tile([128, P2], dt.bfloat16, tag="k1t", name="k1t")
                for j in range(2):
                    cols = slice(j * CH, (j + 1) * CH)
                    nc.vector.scalar_tensor_tensor(
                        out=k1t[:, cols], in0=kps[:, cols], scalar=0.0,
                        in1=mnk[:, cols], op0=OP.max, op1=OP.add,
                        accum_out=kpart[:, 8 * j + c: 8 * j + c + 1])

                # v evac -> vT channel-major (vsum rides ACT accum)
                for j in range(2):
                    nc.scalar.activation(
                        vT[j][:, 1 + c * CH: 1 + (c + 1) * CH],
                        vps[:, j * CH:(j + 1) * CH], AF.Copy,
                        accum_out=vpart[:, 8 * j + c: 8 * j + c + 1])

                # rope(k): ks = k1t*cos + (R.T@k1t)*sin
                m1 = work.tile([128, P2], dt.bfloat16, tag="m1", name="m1")
                nc.gpsimd.tensor_mul(
                    m1[:, :].rearrange("p (r t) -> p r t", r=2),
                    k1t[:, :].rearrange("p (r t) -> p r t", r=2),
                    csl.unsqueeze(1).to_broadcast((128, 2, CH)))
                m2 = work.tile([128, P2], dt.bfloat16, tag="m2", name="m2")
                for j in range(2):
                    cols = slice(j * CH, (j + 1) * CH)
                    rkp = ppool.tile([128, CH], dt.float32, tag="tp", bufs=3,
                                     name="rkp")
                    nc.tensor.matmul(rkp, rblk, k1t[:, cols],
                                     start=True, stop=True)
                    nc.vector.tensor_mul(m2[:, cols], rkp, ssl)
                ks = work.tile([128, P2], dt.bfloat16, tag="ks", name="ks")
                nc.vector.tensor_add(ks, m1, m2)

                # transposes to token-major; kv gram accumulation
                for s in range(4):
                    ktp = ppool.tile([128, CH], dt.bfloat16, tag="tp", bufs=3,
                                     name="ktp")
                    nc.tensor.transpose(ktp[:, 0:128],
                                        ks[:, s * 128:(s + 1) * 128], id16)
                    nc.tensor.transpose(ktp[:, 128:256],
                                        ks[:, CH + s * 128:CH + (s + 1) * 128],
                                        id16)
                    vcol = 1 + c * CH + s * 128
                    nc.tensor.transpose(ktp[:, 256:384],
                                        vT[0][:, vcol:vcol + 128], id16)
                    nc.tensor.transpose(ktp[:, 384:512],
                                        vT[1][:, vcol:vcol + 128], id16)
                    kvtok = work.tile([128, CH], dt.bfloat16, tag="kvtok",
                                      name="kvtok")
                    if s % 2 == 0:
                        nc.scalar.activation(kvtok, ktp, AF.Copy)
                    else:
                        nc.vector.tensor_copy(kvtok, ktp)
                    first = (c == 0 and s == 0)
                    last = (c == NCH - 1 and s == 3)
                    nc.tensor.matmul(gram[:, 0:128], kvtok[:, 0:128],
                                     kvtok[:, 256:384], start=first, stop=False)
                    nc.tensor.matmul(gram[:, 128:256], kvtok[:, 128:256],
                                     kvtok[:, 384:512], start=False, stop=last)

            # ---- halo v columns (tokens t0-1 and t0+T) for the conv
            vhp = ppool.tile([128, CH], dt.float32, tag="tp", bufs=3, name="vhp")
            for j in range(2):
                vsl = slice(256 + 128 * j, 256 + 128 * (j + 1))
                cl = slice(j * 4, j * 4 + 1)
                cr = slice(j * 4 + 2, j * 4 + 3)
                nc.tensor.matmul(vhp[:, cl], wkv[0][:, vsl], xct[0][:, 0:1],
                                 start=(j == 0), stop=False)
                nc.tensor.matmul(vhp[:, cl], wkv[1][:, vsl], xct[1][:, 0:1],
                                 start=False, stop=False)
                nc.tensor.matmul(vhp[:, cr], wkv[0][:, vsl], xct[0][:, TH - 1:TH],
                                 start=False, stop=False)
                nc.tensor.matmul(vhp[:, cr], wkv[1][:, vsl], xct[1][:, TH - 1:TH],
                                 start=False, stop=(j == 1))
            for j in range(2):
                nc.scalar.activation(vT[j][:, 0:1], vhp[:, j * 4:j * 4 + 1],
                                     AF.Copy)
                nc.scalar.activation(vT[j][:, TH - 1:TH],
                                     vhp[:, j * 4 + 2:j * 4 + 3], AF.Copy)

            # ======================= stats + allreduce =====================
            nc.vector.tensor_scalar_mul(stats[:, 0:256], gram, 1.0)
            nc.vector.tensor_reduce(stats[:, 256:257], kpart[:, 0:8],
                                    axis=mybir.AxisListType.X, op=OP.add)
            nc.vector.tensor_reduce(stats[:, 257:258], kpart[:, 8:16],
                                    axis=mybir.AxisListType.X, op=OP.add)
            nc.vector.tensor_reduce(stats[:, 258:259], vpart[:, 0:8],
                                    axis=mybir.AxisListType.X, op=OP.add)
            nc.vector.tensor_reduce(stats[:, 259:260], vpart[:, 8:16],
                                    axis=mybir.AxisListType.X, op=OP.add)

            if nocc:
                nc.vector.tensor_scalar_mul(stats2, stats, 1.0)
            else:
                ccin = dpool.tile([128, 260], dt.float32, tag="ccin",
                                  name="ccin")
                ccout = dpool.tile([128, 260], dt.float32, tag="ccout",
                                   name="ccout")
                nc.gpsimd.dma_start(out=ccin[:, :], in_=stats)
                nc.gpsimd.collective_compute(
                    "AllReduce", OP.add, replica_groups=RG,
                    ins=[ccin[:, :]], outs=[ccout[:, :]])
                nc.gpsimd.dma_start(out=stats2, in_=ccout[:, :])

            # =========================== phase Q ===========================
            # q/o projections + rope(q).  Independent of the collective ->
            # overlaps it.  No gpsimd here (its queue waits on the AR).
            for c in range(NCH):
                xsl = [x[:, 1 + c * CH: 1 + (c + 1) * CH] for x in xct]
                psl = slice(c * P2, (c + 1) * P2)
                ssl = srep[:, c * CH:(c + 1) * CH]
                csl = crep[:, c * CH:(c + 1) * CH]

                qps = ppool.tile([128, P2], dt.float32, tag="big", name="qps")
                ops_ = ppool.tile([128, P2], dt.float32, tag="big", name="ops_")
                for j in range(2):
                    cols = slice(j * CH, (j + 1) * CH)
                    msl = slice(128 * j, 128 * (j + 1))
                    nc.tensor.matmul(qps[:, cols], wq[0][:, msl], xsl[0],
                                     start=True, stop=False)
                    nc.tensor.matmul(qps[:, cols], wq[1][:, msl], xsl[1],
                                     start=False, stop=not use_bias)
                    if use_bias:
                        nc.tensor.matmul(qps[:, cols], bqkvo[:, msl], ones,
                                         start=False, stop=True)
                    nc.tensor.matmul(ops_[:, cols], wo[0][:, msl], xsl[0],
                                     start=True, stop=False)
                    nc.tensor.matmul(ops_[:, cols], wo[1][:, msl], xsl[1],
                                     start=False, stop=not use_bias)
                    if use_bias:
                        nc.tensor.matmul(ops_[:, cols],
                                         bqkvo[:, 768 + 128 * j:768 + 128 * (j + 1)],
                                         ones, start=False, stop=True)

                # elu(q)+1 -> q1p
                eq = work.tile([128, P2], dt.bfloat16, tag="ek", name="eq")
                nc.scalar.activation(eq, qps, AF.Exp)
                mnq = work.tile([128, P2], dt.bfloat16, tag="mnk", name="mnq")
                nc.vector.tensor_scalar_min(mnq, eq, 1.0)
                nc.vector.scalar_tensor_tensor(
                    out=q1p[:, psl], in0=qps, scalar=0.0, in1=mnq,
                    op0=OP.max, op1=OP.add)

                # o evac
                nc.scalar.activation(o1p[:, psl], ops_, AF.Copy)

                # rope(q) -> qs0  (all on DVE; gpsimd queue is busy w/ AR)
                m1 = work.tile([128, P2], dt.bfloat16, tag="m1", name="m1q")
                nc.vector.tensor_mul(
                    m1[:, :].rearrange("p (r t) -> p r t", r=2),
                    q1p[:, psl].rearrange("p (r t) -> p r t", r=2),
                    csl.unsqueeze(1).to_broadcast((128, 2, CH)))
                m2 = work.tile([128, P2], dt.bfloat16, tag="m2", name="m2q")
                for j in range(2):
                    cols = slice(j * CH, (j + 1) * CH)
                    rqp = ppool.tile([128, CH], dt.float32, tag="tp", bufs=3,
                                     name="rqp")
                    nc.tensor.matmul(rqp, rblk,
                                     q1p[:, c * P2 + j * CH: c * P2 + (j + 1) * CH],
                                     start=True, stop=True)
                    nc.vector.tensor_mul(m2[:, cols], rqp, ssl)
                nc.vector.tensor_add(qs0[:, psl], m1, m2)

            # =========================== phase Z ===========================
            zsc = const.tile([128, 2], dt.float32, tag="zsc", name="zsc")
            nc.scalar.activation(zsc, stats2[:, 256:258], AF.Copy,
                                 scale=float(SCALE / N))
            zblk = []
            kvblk = []
            mcorr = []
            for j in range(2):
                zb = const.tile([128, 128], dt.bfloat16, tag=f"zblk{j}",
                                name=f"zblk{j}")
                nc.vector.tensor_tensor(
                    zb, zsc[:, j:j + 1].to_broadcast((128, 128)), hmask,
                    OP.mult)
                zblk.append(zb)
                kvb = const.tile([128, 128], dt.bfloat16, tag=f"kvb{j}",
                                 name=f"kvb{j}")
                nc.vector.tensor_tensor(
                    kvb, stats2[:, 128 * j:128 * (j + 1)], hmk, OP.mult)
                kvblk.append(kvb)

            # vmean row: cast (scaled by -1/N) -> transpose -> broadcast
            vrin = const.tile([128, 2], dt.bfloat16, tag="vrin", name="vrin")
            nc.scalar.activation(vrin, stats2[:, 258:260], AF.Copy,
                                 scale=float(-1.0 / N))
            vtp = ppool.tile([128, CH], dt.bfloat16, tag="tp", bufs=3,
                             name="vtp")
            for j in range(2):
                nc.tensor.transpose(vtp[0:1, 128 * j:128 * (j + 1)],
                                    vrin[:, j:j + 1], id16)
            vrow = const.tile([1, 256], dt.bfloat16, tag="vrow", name="vrow")
            nc.scalar.activation(vrow, vtp[0:1, 0:256], AF.Copy)
            for j in range(2):
                vrb = const.tile([128, 128], dt.bfloat16, tag=f"vrb{j}",
                                 name=f"vrb{j}")
                nc.gpsimd.partition_broadcast(vrb, vrow[0:1, 128 * j:128 * (j + 1)])
                mc = const.tile([128, 128], dt.bfloat16, tag=f"mc{j}",
                                name=f"mc{j}")
                nc.vector.tensor_tensor(mc, zblk[j], vrb, OP.mult)
                mcorr.append(mc)

            # =========================== phase C ===========================
            for c in range(NCH):
                psl = slice(c * P2, (c + 1) * P2)

                zps = ppool.tile([128, P2], dt.float32, tag="big", name="zps")
                for j in range(2):
                    cols = slice(j * CH, (j + 1) * CH)
                    nc.tensor.matmul(
                        zps[:, cols], zblk[j],
                        q1p[:, c * P2 + j * CH: c * P2 + (j + 1) * CH],
                        start=True, stop=True)
                rz = work.tile([128, P2], dt.float32, tag="rz", name="rz")
                nc.vector.reciprocal_approx_fast(out=rz, in_=zps)
                qa = work.tile([128, P2], dt.bfloat16, tag="qa", name="qa")
                nc.vector.scalar_tensor_tensor(
                    out=qa, in0=rz, scalar=1.0, in1=qs0[:, psl],
                    op0=OP.add, op1=OP.mult)

                rps = ppool.tile([128, P2], dt.float32, tag="big", name="rps")
                for j in range(2):
                    cols = slice(j * CH, (j + 1) * CH)
                    nc.tensor.matmul(rps[:, cols], kvblk[j], qa[:, cols],
                                     start=True, stop=False)
                    nc.tensor.matmul(rps[:, cols], mcorr[j],
                                     q1p[:, c * P2 + j * CH: c * P2 + (j + 1) * CH],
                                     start=False, stop=False)
                    for tap in range(3):
                        lastmm = (tap == 2 and not use_bias)
                        nc.tensor.matmul(
                            rps[:, cols],
                            dcw[:, (tap * 2 + j) * 128:(tap * 2 + j + 1) * 128],
                            vT[j][:, c * CH + tap: c * CH + tap + CH],
                            start=False, stop=lastmm)
                    if use_bias:
                        nc.tensor.matmul(rps[:, cols],
                                         blep[:, 128 * j:128 * (j + 1)],
                                         ones, start=False, stop=True)

                y = work.tile([128, P2], dt.bfloat16, tag="y", name="y")
                nc.vector.tensor_mul(y, rps, o1p[:, psl])

                outp = ppool.tile([128, P2], dt.float32, tag="big", name="outp")
                for s in range(4):
                    osl = slice(s * 256, (s + 1) * 256)
                    nc.tensor.matmul(
                        outp[:, osl], y[:, s * 128: (s + 1) * 128],
                        wp[0], start=True, stop=False)
                    nc.tensor.matmul(
                        outp[:, osl], y[:, CH + s * 128: CH + (s + 1) * 128],
                        wp[1], start=False, stop=not use_bias)
                    if use_bias:
                        nc.tensor.matmul(outp[:, osl], ones[:, 0:128], bprj,
                                         start=False, stop=True)
                outsb = work.tile([128, P2], dt.bfloat16, tag="outsb",
                                  name="outsb")
                nc.scalar.activation(outsb, outp, AF.Copy)
                dsl = out_d[c * CH: (c + 1) * CH, :]
                nc.sync.dma_start(out=dsl.rearrange("(s t) o -> t s o", s=4),
                                  in_=outsb)

    nc.compile()
    return nc


_NC_CACHE = {}


def _get_nc(use_bias: bool):
    nocc = bool(os.environ.get("KERNEL_NOCC"))
    key = (use_bias, nocc)
    if key not in _NC_CACHE:
        _NC_CACHE[key] = _build_nc(use_bias, nocc)
    return _NC_CACHE[key]


def kernel(x, sin, cos, W_qkvo, b_qkvo, W_lepe, b_lepe, W_proj, b_proj):
    from concourse.bass_utils import run_bass_kernel_spmd

    per_core, use_bias = _host_prep(x, sin, cos, W_qkvo, b_qkvo, W_lepe,
                                    b_lepe, W_proj, b_proj)
    nc = _get_nc(use_bias)
    # keep only the inputs that survived DCE in the compiled program
    import concourse.mybir as mybir
    expected = set()
    for alloc in nc.m.functions[0].allocations:
        if isinstance(alloc, mybir.MemoryLocationSet) and alloc.kind == "ExternalInput":
            expected.add(alloc.memorylocations[0].name)
    per_core = [{k: v for k, v in m.items() if k in expected} for m in per_core]
    res = run_bass_kernel_spmd(nc, per_core, core_ids=list(range(NCORES)),
                               trace=bool(os.environ.get("KERNEL_TRACE")))
    if os.environ.get("KERNEL_TRACE"):
        kernel.last_exec_time_ns = res.exec_time_ns
        kernel.last_results = res
    full = np.zeros((B, N, INTERNAL), np.float32)
    for c in range(NCORES):
        b = c // 2
        t0 = (c % 2) * T
        full[b, t0:t0 + T] = res.results[c]["out"].astype(np.float32)
    return full


# ---------------------------------------------------------- numpy reference
# A numpy emulation of the device pipeline (fp32), used to validate the
# decomposition (run with KERNEL_SELFTEST=1).

def _numpy_pipeline(per_core_inputs, skip_pair=False):
    outs = []
    cores = []
    for c in range(NCORES):
        d = per_core_inputs[c]
        xct = d["xct"].astype(np.float32)          # [256, TH]
        srep = d["srep"].astype(np.float32)
        crep = d["crep"].astype(np.float32)
        wq = d["wq"].astype(np.float32)
        wkv = d["wkv"].astype(np.float32)
        wo = d["wo"].astype(np.float32)
        wp = d["wp"].astype(np.float32)
        dcw = d["dcw"].astype(np.float32).reshape(128, 6, 128)
        R = d["rblk"].astype(np.float32)
        hmask = d["hmask"].astype(np.float32)

        x_in = xct[:, 1:T + 1]                     # [256, T]
        qT = wq.T @ x_in                           # [256, T]
        kT = wkv[:, 0:256].T @ x_in
        vT_m = wkv[:, 256:512].T @ x_in
        oT = wo.T @ x_in
        # halo v cols
        vhl = wkv[:, 256:512].T @ xct[:, 0:1]
        vhr = wkv[:, 256:512].T @ xct[:, TH - 1:TH]
        vT = np.concatenate([vhl, vT_m, vhr], axis=1)      # [256, TH]

        def elu1(t):
            return np.minimum(np.exp(t), 1.0) + np.maximum(t, 0.0)

        q1 = elu1(qT)
        k1 = elu1(kT)

        # K rope (per chan-tile with R)
        ks = np.zeros_like(k1)
        qs = np.zeros_like(q1)
        for j in range(2):
            blk = k1[128 * j:128 * (j + 1)]
            ks[128 * j:128 * (j + 1)] = blk * crep + (R.T @ blk) * srep
            qb = q1[128 * j:128 * (j + 1)]
            qs[128 * j:128 * (j + 1)] = qb * crep + (R.T @ qb) * srep

        # kv gram per tile: ks_j^T tokens x v_j
        gram = np.zeros((128, 256), np.float32)
        for j in range(2):
            gram[:, 128 * j:128 * (j + 1)] = (
                ks[128 * j:128 * (j + 1)] @ vT[128 * j:128 * (j + 1), 1:T + 1].T)
        ksum = k1.sum(axis=1)                      # [256]
        vsum = vT[:, 1:T + 1].sum(axis=1)
        cores.append(dict(d=d, q1=q1, qs=qs, oT=oT, vT=vT, gram=gram,
                          ksum=ksum, vsum=vsum, R=R, hmask=hmask, dcw=dcw,
                          wp=wp))

    for pair in range(4):
        a, b2 = cores[2 * pair], cores[2 * pair + 1]
        if skip_pair:
            for cc in (a, b2):
                cc["gram_r"], cc["ksum_r"], cc["vsum_r"] = (
                    cc["gram"], cc["ksum"], cc["vsum"])
            continue
        gram = a["gram"] + b2["gram"]
        ksum = a["ksum"] + b2["ksum"]
        vsum = a["vsum"] + b2["vsum"]
        for cc in (a, b2):
            cc["gram_r"], cc["ksum_r"], cc["vsum_r"] = gram, ksum, vsum

    for c in range(NCORES):
        st = cores[c]
        q1, qs, oT, vT = st["q1"], st["qs"], st["oT"], st["vT"]
        hmask, dcw, wp = st["hmask"], st["dcw"], st["wp"]
        gram, ksum, vsum = st["gram_r"], st["ksum_r"], st["vsum_r"]

        kmean = ksum / N
        vmean = vsum / N
        res = np.zeros((256, T), np.float32)
        for j in range(2):
            sl = slice(128 * j, 128 * (j + 1))
            zsc = SCALE * kmean[sl]                          # [128]
            zblk = (zsc[:, None] * hmask)                    # [128,128]
            zrep = zblk.T @ q1[sl]                           # [128, T]
            r = 1.0 / zrep
            qa = qs[sl] * (1.0 + r)
            kvblk = KSC * gram[:, 128 * j:128 * (j + 1)] * hmask
            mcorr = -(zsc[:, None]) * vmean[sl][None, :] * hmask
            lepe = np.zeros((128, T), np.float32)
            for tap in range(3):
                dw = dcw[:, tap * 2 + j, :]
                lepe += dw.T @ vT[sl, tap:tap + T]
            res[sl] = (kvblk.T @ qa + mcorr.T @ q1[sl] + lepe)
        y = res * oT
        out = y.T @ wp
        outs.append(out.astype(np.float32))

    # unshard
    full = np.zeros((B, N, 256), np.float32)
    for c in range(NCORES):
        b = c // 2
        t0 = (c % 2) * T
        full[b, t0:t0 + T] = outs[c]
    return full


if __name__ == "__main__" and os.environ.get("KERNEL_BUILD"):
    nc = _build_nc(False)
    import tempfile
    from concourse.bass_utils import compile_bass_kernel
    print("NEFF:", compile_bass_kernel(nc, tempfile.mkdtemp()))

if __name__ == "__main__" and os.environ.get("KERNEL_SELFTEST"):
    sys.path.insert(0, os.path.dirname(os.path.abspath(__file__)))
    import reference
    inputs = reference.setup_inputs()
    inputs = {k: np.asarray(v) for k, v in inputs.items()}
    expected = np.asarray(reference.reference(**inputs))
    per_core, use_bias = _host_prep(**inputs)
    got = _numpy_pipeline(per_core)
    err = np.abs(got - expected)
    rel = np.linalg.norm(got - expected) / np.linalg.norm(expected)
    print("selftest rel err:", rel, "max abs:", err.max())
